# revision 1
# baseline (speedup 1.0000x reference)
"""APPNP GNN message passing on 8 TRN2 NeuronCores.

Self-contained: takes full inputs, shards internally, runs a Bass SPMD
kernel (AllGather per propagation step + SWDGE gather for the edge
message passing; bulk edges through a fixed per-(dst,chunk) slot grid
reduced on DVE, overflow edges through a one-hot matmul reduce on PE),
returns the full [100000, 64] output.
"""
import sys
sys.path.insert(0, '/opt/trn_rl_repo')
import numpy as np

from concourse import bacc, bass, tile
from concourse.bass_utils import run_bass_kernel_spmd

mybir = bass.mybir
f32 = mybir.dt.float32
i16 = mybir.dt.int16
Alu = mybir.AluOpType
ActF = mybir.ActivationFunctionType
Axis = mybir.AxisListType


# problem constants (hardcoded per spec); override via configure() for debug
def configure(N=100000, F=512, HID=64, C=64, ALPHA=0.1, NITER=10, P=8,
              CAP=10, SBW=4, G=4, FEAT=('xform', 'iters', 'dve', 'fix', 'smx')):
    g = globals()
    g['N'], g['F'], g['HID'], g['C'] = N, F, HID, C
    g['ALPHA'], g['NITER'], g['P'] = ALPHA, NITER, P
    g['CAP'], g['SBW'], g['G'] = CAP, SBW, G
    g['FEAT'] = set(FEAT)
    g['NSH'] = N // P
    g['NB'] = -(-g['NSH'] // 128)
    g['PSH'] = g['NB'] * 128
    assert (g['PSH'] * P) % G == 0
    g['CHW'] = g['PSH'] * P // G
    assert g['CHW'] <= 32767, "int16 index range"
    g['NSB'] = -(-g['NB'] // SBW)
    g['SEC1'] = 128 * (CAP + 1)          # slots per (g, dst-block): grid + fixup
    g['NSLOT'] = g['NB'] * g['SEC1'] * G
    _NC_CACHE.clear()


_NC_CACHE = {}
configure()


def _sb_geom(sb):
    nB = SBW if sb < NSB - 1 else NB - SBW * (NSB - 1)
    base = sb * (SBW * SEC1 * G)
    return nB, base


def _build_graph():
    nc = bacc.Bacc(None, target_bir_lowering=False, debug=False, num_devices=P)

    xT = nc.dram_tensor("xT", [F, NSH], f32, kind="ExternalInput")
    W1 = nc.dram_tensor("W1", [F, HID], f32, kind="ExternalInput")
    W2 = nc.dram_tensor("W2", [HID, C], f32, kind="ExternalInput")
    gidx = nc.dram_tensor("gidx", [128, NSLOT // 16], i16, kind="ExternalInput")
    gwd = nc.dram_tensor("gw", [128, NB * G * CAP], f32, kind="ExternalInput")
    iota = nc.dram_tensor("iota", [128, 128], f32, kind="ExternalInput")
    fdl = nc.dram_tensor("fdl", [128, NB * G], f32, kind="ExternalInput")
    fwl = nc.dram_tensor("fwl", [128, NB * G], f32, kind="ExternalInput")
    out_p = nc.dram_tensor("out", [NSH, C], f32, kind="ExternalOutput")

    rg = [list(range(P))]

    with tile.TileContext(nc) as tc:
        with tc.tile_pool(name="dram", bufs=1, space="DRAM") as dram, \
             tc.tile_pool(name="const", bufs=1) as cp:
            zag_in = dram.tile([PSH, C], f32)
            zfull = nc.dram_tensor("zfull_shared", [PSH * P, C], f32,
                                   addr_space="Shared")

            # resident constants / metadata
            w1_sb = cp.tile([128, F // 128, HID], f32)
            nc.sync.dma_start(out=w1_sb[:], in_=W1[:].rearrange("(k p) h -> p k h", p=128))
            w2_sb = cp.tile([HID, C], f32)
            nc.sync.dma_start(out=w2_sb[:], in_=W2[:])
            gw_sb = cp.tile([128, NB * G * CAP], f32)
            nc.sync.dma_start(out=gw_sb[:], in_=gwd[:])
            iota_sb = cp.tile([128, 128], f32)
            nc.sync.dma_start(out=iota_sb[:], in_=iota[:])
            fdl_sb = cp.tile([128, NB * G], f32)
            nc.sync.dma_start(out=fdl_sb[:], in_=fdl[:])
            fwl_sb = cp.tile([128, NB * G], f32)
            nc.sync.dma_start(out=fwl_sb[:], in_=fwl[:])

            z0_sb = cp.tile([128, NB, C], f32)   # alpha-source, node-major permuted
            z_sb = cp.tile([128, NB, C], f32)    # per-iteration z shard

            nc.vector.memset(z0_sb[:], 0.0)
            if 'dve' not in FEAT:
                nc.vector.memset(z_sb[:], 0.0)

            # ---- feature transform: hT = relu(x @ W1).T ; z0 = h @ W2 ----
            if 'xform' in FEAT:
              with tc.tile_pool(name="ht", bufs=1) as hp, \
                 tc.tile_pool(name="xf", bufs=3) as xf, \
                 tc.tile_pool(name="ps", bufs=2, space="PSUM") as ps:
                  hT_sb = hp.tile([HID, NSH], f32)
                  JW = 512
                  for j0 in range(0, NSH, JW):
                      jw = min(JW, NSH - j0)
                      xsb = xf.tile([128, F // 128, JW], f32, tag="xsb")
                      nc.sync.dma_start(
                          out=xsb[:, :, :jw],
                          in_=xT[:, j0:j0 + jw].rearrange("(k p) n -> p k n", p=128))
                      hps = ps.tile([HID, JW], f32, tag="hps")
                      for k in range(F // 128):
                          nc.tensor.matmul(hps[:, :jw], lhsT=w1_sb[:, k, :],
                                           rhs=xsb[:, k, :jw],
                                           start=(k == 0), stop=(k == F // 128 - 1))
                      nc.scalar.activation(out=hT_sb[:, j0:j0 + jw], in_=hps[:, :jw],
                                           func=ActF.Relu)
                  for B in range(NB):
                      pw = 128 if B < NB - 1 else NSH - 128 * (NB - 1)
                      zps = ps.tile([128, C], f32, tag="zps")
                      nc.tensor.matmul(zps[:pw, :], lhsT=hT_sb[:, B * 128:B * 128 + pw],
                                       rhs=w2_sb[:], start=True, stop=True)
                      nc.scalar.copy(out=z0_sb[:pw, B, :], in_=zps[:pw, :])

            # iteration 1 input
            nc.sync.dma_start(out=zag_in[:], in_=z0_sb[:])

            # ---- propagation iterations ----
            with tc.tile_pool(name="msgp", bufs=2) as mp, \
                 tc.tile_pool(name="idxp", bufs=4) as ip, \
                 tc.tile_pool(name="redp", bufs=4) as rp, \
                 tc.tile_pool(name="ohp", bufs=4) as ohp, \
                 tc.tile_pool(name="pf", bufs=2, space="PSUM") as pf:
                for it in range(NITER if 'iters' in FEAT else 0):
                    nc.gpsimd.collective_compute(
                        "AllGather", Alu.bypass, replica_groups=rg,
                        ins=[zag_in[:]], outs=[zfull[:]])

                    for sb in range(NSB):
                        nB, base = _sb_geom(sb)
                        msg = mp.tile([128, SBW * G * (CAP + 1), C], f32, tag="msg")
                        for g in range(G):
                            nsl = nB * SEC1
                            w16 = nsl // 16
                            ic = ip.tile([128, SBW * SEC1 // 16], i16, tag="ic")
                            col0 = (base + g * nsl) // 16
                            nc.sync.dma_start(out=ic[:, :w16],
                                              in_=gidx[:, col0:col0 + w16])
                            nc.gpsimd.dma_gather(
                                out_ap=msg[:, g * (nB * (CAP + 1)):(g + 1) * (nB * (CAP + 1)), :],
                                in_ap=zfull[g * CHW:(g + 1) * CHW, :],
                                idxs_ap=ic[:, :w16],
                                num_idxs=nsl, num_idxs_reg=nsl, elem_size=C,
                                single_packet=False)
                        mfull = msg[:, :G * nB * (CAP + 1), :].rearrange(
                            "p (g x) c -> p g x c", g=G)
                        grid = mfull[:, :, :nB * CAP, :].rearrange(
                            "p g (b k) c -> p g b k c", k=CAP)
                        for bl in range(nB):
                            if 'dve' not in FEAT:
                                continue
                            B = sb * SBW + bl
                            m_b = grid[:, :, bl]
                            wv = gw_sb[:, B * G * CAP:(B + 1) * G * CAP].rearrange(
                                "p (g k) -> p g k", g=G).unsqueeze(3).broadcast_to(
                                [128, G, CAP, C])
                            nc.vector.scalar_tensor_tensor(
                                out=m_b, in0=m_b, scalar=1.0, in1=wv,
                                op0=Alu.mult, op1=Alu.mult)
                            red = rp.tile([128, C], f32, tag="red")
                            nc.vector.tensor_reduce(
                                out=red[:], in_=m_b.transpose([0, 3, 1, 2]),
                                axis=Axis.XY, op=Alu.add)
                            nc.vector.scalar_tensor_tensor(
                                out=z_sb[:, B, :], in0=z0_sb[:, B, :], scalar=ALPHA,
                                in1=red[:], op0=Alu.mult, op1=Alu.add)
                        # overflow edges: one-hot matmul reduce on PE
                        if 'fix' not in FEAT:
                            continue
                        psF = pf.tile([128, SBW, C], f32, tag="psF")
                        for bl in range(nB):
                            B = sb * SBW + bl
                            for g in range(G):
                                oh = ohp.tile([128, 128], f32, tag="oh")
                                col = B * G + g
                                nc.vector.tensor_scalar(
                                    oh[:], iota_sb[:],
                                    fdl_sb[:, col:col + 1], fwl_sb[:, col:col + 1],
                                    Alu.is_equal, Alu.mult)
                                nc.tensor.matmul(
                                    psF[:, bl, :], lhsT=oh[:],
                                    rhs=mfull[:, g, nB * CAP + bl, :],
                                    start=(g == 0), stop=(g == G - 1))
                        for bl in range(nB):
                            B = sb * SBW + bl
                            nc.vector.scalar_tensor_tensor(
                                out=z_sb[:, B, :], in0=z_sb[:, B, :], scalar=1.0,
                                in1=psF[:, bl, :], op0=Alu.mult, op1=Alu.add)

                    nc.sync.dma_start(out=zag_in[:], in_=z_sb[:])

                # ---- log_softmax on final z ----
                with tc.tile_pool(name="smx", bufs=2) as sp:
                  if 'smx' in FEAT:
                    mneg = cp.tile([128, NB], f32)
                    nc.vector.tensor_reduce(out=mneg[:], in_=z_sb[:],
                                            axis=Axis.X, op=Alu.max)
                    nc.vector.tensor_scalar_mul(mneg[:], mneg[:], -1.0)
                    ssum = cp.tile([128, NB], f32)
                    for B in range(NB):
                        e_scr = sp.tile([128, C], f32, tag="escr")
                        nc.scalar.activation(out=e_scr[:], in_=z_sb[:, B, :],
                                             func=ActF.Exp, bias=mneg[:, B:B + 1],
                                             accum_out=ssum[:, B:B + 1])
                    lneg = cp.tile([128, NB], f32)
                    nc.scalar.activation(out=lneg[:], in_=ssum[:], func=ActF.Ln)
                    nc.vector.tensor_scalar_mul(lneg[:], lneg[:], -1.0)
                    for B in range(NB):
                        nc.vector.tensor_scalar(
                            z0_sb[:, B, :], z_sb[:, B, :],
                            mneg[:, B:B + 1], lneg[:, B:B + 1],
                            Alu.add, Alu.add)

                    nc.sync.dma_start(
                        out=out_p[0:128 * (NB - 1), :].rearrange(
                            "(b p) c -> p b c", p=128),
                        in_=z0_sb[:, 0:NB - 1, :])
                    lastw = NSH - 128 * (NB - 1)
                    nc.sync.dma_start(out=out_p[128 * (NB - 1):NSH, :],
                                      in_=z0_sb[0:lastw, NB - 1, :])

    nc.finalize()
    return nc


def _prow(n):
    """global node id -> permuted z-table row"""
    s, loc = n // NSH, n % NSH
    return s * PSH + (loc % 128) * NB + loc // 128


def _rank_within(key):
    """rank of each element within its group (key values arbitrary)."""
    order = np.argsort(key, kind='stable')
    ks = key[order]
    if len(ks) == 0:
        return order, np.zeros(0, np.int64)
    starts = np.concatenate([[True], ks[1:] != ks[:-1]])
    gid = np.cumsum(starts) - 1
    first = np.full(gid[-1] + 1, len(ks), np.int64)
    np.minimum.at(first, gid, np.arange(len(ks)))
    rank = np.arange(len(ks)) - first[gid]
    return order, rank


def _preprocess(x, edge_index, edge_weight):
    src = edge_index[0].astype(np.int64)
    dst = edge_index[1].astype(np.int64)
    w = edge_weight.astype(np.float32)

    sp = _prow(src)
    g_e = sp // CHW
    lidx = (sp % CHW).astype(np.int64)
    core = dst // NSH
    dloc = dst % NSH

    cell = (core * NSH + dloc) * G + g_e
    order, rank = _rank_within(cell)
    do, wo, go, lo, co = (a[order] for a in (dloc, w, g_e, lidx, core))
    ing = rank < CAP

    B = do // 128
    p = do % 128
    sbb = np.minimum(B // SBW, NSB - 1)
    b_loc = B - sbb * SBW
    nB_arr = np.where(sbb < NSB - 1, SBW, NB - SBW * (NSB - 1))
    sb_base = sbb * (SBW * SEC1 * G)
    sec_base = sb_base + go * (nB_arr * SEC1)

    gidx_all = np.zeros((P, NSLOT), np.int16)
    gw_all = np.zeros((P, 128, NB * G * CAP), np.float32)
    m = ing
    i_slot = sec_base[m] + (b_loc[m] * CAP + rank[m]) * 128 + p[m]
    gidx_all[co[m], i_slot] = lo[m].astype(np.int16)
    gw_all[co[m], p[m], B[m] * G * CAP + go[m] * CAP + rank[m]] = wo[m]

    # overflow edges -> fixup slots: one 128-slot block per (B, g)
    fdl_all = np.full((P, 128, NB * G), -1.0, np.float32)
    fwl_all = np.zeros((P, 128, NB * G), np.float32)
    ov = ~ing
    okey = (co[ov] * NB + B[ov]) * G + go[ov]
    oord, orank = _rank_within(okey)
    assert orank.size == 0 or orank.max() < 128, f"fixup overflow {orank.max()}"
    o_c, o_B, o_g = co[ov][oord], B[ov][oord], go[ov][oord]
    o_bl, o_sb = b_loc[ov][oord], sbb[ov][oord]
    o_secb = sec_base[ov][oord]
    o_nB = nB_arr[ov][oord]
    fi_slot = o_secb + (o_nB * CAP + o_bl) * 128 + orank
    gidx_all[o_c, fi_slot] = lo[ov][oord].astype(np.int16)
    fdl_all[o_c, orank, o_B * G + o_g] = (do[ov][oord] % 128).astype(np.float32)
    fwl_all[o_c, orank, o_B * G + o_g] = wo[ov][oord]

    gidx_w = np.tile(gidx_all.reshape(P, NSLOT // 16, 16).transpose(0, 2, 1),
                     (1, 8, 1))
    return gidx_w, gw_all, fdl_all, fwl_all


def _make_in_maps(x, edge_index, edge_weight, W1, W2):
    gidx_w, gw_all, fdl_all, fwl_all = _preprocess(x, edge_index, edge_weight)
    iota = np.broadcast_to(np.arange(128, dtype=np.float32), (128, 128)).copy()
    in_maps = []
    for c in range(P):
        in_maps.append({
            "xT": np.ascontiguousarray(x[c * NSH:(c + 1) * NSH].T),
            "W1": W1, "W2": W2,
            "gidx": np.ascontiguousarray(gidx_w[c]),
            "gw": np.ascontiguousarray(gw_all[c]),
            "iota": iota,
            "fdl": np.ascontiguousarray(fdl_all[c]),
            "fwl": np.ascontiguousarray(fwl_all[c]),
        })
    return in_maps


def kernel(x, edge_index, edge_weight, W1, W2):
    x = np.ascontiguousarray(np.asarray(x, np.float32))
    edge_index = np.asarray(edge_index, np.int32)
    edge_weight = np.asarray(edge_weight, np.float32)
    W1 = np.ascontiguousarray(np.asarray(W1, np.float32))
    W2 = np.ascontiguousarray(np.asarray(W2, np.float32))

    if 'nc' not in _NC_CACHE:
        _NC_CACHE['nc'] = _build_graph()
    nc = _NC_CACHE['nc']

    in_maps = _make_in_maps(x, edge_index, edge_weight, W1, W2)
    res = run_bass_kernel_spmd(nc, in_maps, core_ids=list(range(P)))
    out = np.concatenate([res.results[c]["out"] for c in range(P)], axis=0)
    return out.astype(np.float32)



# revision 7
# speedup vs baseline: 2.3210x; 2.3210x over previous
"""APPNP GNN message passing on 8 TRN2 NeuronCores.

Self-contained: takes full inputs, shards internally, runs a Bass SPMD
kernel (AllGather per propagation step + SWDGE gather for the edge
message passing; bulk edges through a fixed per-(dst,chunk) slot grid
reduced on DVE, overflow edges through a one-hot matmul reduce on PE),
returns the full [100000, 64] output.

Perf notes: gathers are issued round-robin on 4 SWDGE queues (descriptor
generation runs concurrently per queue, ~3.3x faster than one queue);
fixup one-hot matrices are precomputed on host and streamed from DRAM
instead of being built per-iteration on DVE.
"""
import sys
sys.path.insert(0, '/opt/trn_rl_repo')
import numpy as np

from concourse import bacc, bass, tile
from concourse.bass_utils import run_bass_kernel_spmd

mybir = bass.mybir
f32 = mybir.dt.float32
i16 = mybir.dt.int16
Alu = mybir.AluOpType
ActF = mybir.ActivationFunctionType
Axis = mybir.AxisListType


# problem constants (hardcoded per spec); override via configure() for debug
def configure(N=100000, F=512, HID=64, C=64, ALPHA=0.1, NITER=10, P=8,
              CAP=10, SBW=4, G=4, FEAT=('xform', 'iters', 'dve', 'fix', 'smx')):
    g = globals()
    g['N'], g['F'], g['HID'], g['C'] = N, F, HID, C
    g['ALPHA'], g['NITER'], g['P'] = ALPHA, NITER, P
    g['CAP'], g['SBW'], g['G'] = CAP, SBW, G
    g['FEAT'] = set(FEAT)
    g['NSH'] = N // P
    g['NB'] = -(-g['NSH'] // 128)
    g['PSH'] = g['NB'] * 128
    assert (g['PSH'] * P) % G == 0
    g['CHW'] = g['PSH'] * P // G
    assert g['CHW'] <= 32767, "int16 index range"
    g['NSB'] = -(-g['NB'] // SBW)
    g['SEC1'] = 128 * (CAP + 1)          # slots per (g, dst-block): grid + fixup
    g['NSLOT'] = g['NB'] * g['SEC1'] * G
    _NC_CACHE.clear()


_NC_CACHE = {}
configure()


def _sb_geom(sb):
    nB = SBW if sb < NSB - 1 else NB - SBW * (NSB - 1)
    base = sb * (SBW * SEC1 * G)
    return nB, base


def _build_graph():
    nc = bacc.Bacc(None, target_bir_lowering=False, debug=False, num_devices=P,
                   num_swdge_queues=4)

    xT = nc.dram_tensor("xT", [F, NSH], f32, kind="ExternalInput")
    W1 = nc.dram_tensor("W1", [F, HID], f32, kind="ExternalInput")
    W2 = nc.dram_tensor("W2", [HID, C], f32, kind="ExternalInput")
    gidx = nc.dram_tensor("gidx", [128, NSLOT // 16], i16, kind="ExternalInput")
    # per-slot weights incl. fixup slot (weight 1.0): [128, NB*G*(CAP+1)]
    gwd = nc.dram_tensor("gw", [128, NB * G * (CAP + 1)], f32,
                         kind="ExternalInput")
    # fixup one-hot matrices, host-precomputed: [128, NB*G*128]
    fohd = nc.dram_tensor("foh", [128, NB * G * 128], f32, kind="ExternalInput")
    out_p = nc.dram_tensor("out", [NSH, C], f32, kind="ExternalOutput")

    rg = [list(range(P))]

    with tile.TileContext(nc) as tc:
        with tc.tile_pool(name="dram", bufs=1, space="DRAM") as dram, \
             tc.tile_pool(name="const", bufs=1) as cp:
            zag_in = dram.tile([PSH, C], f32)
            zfull2 = [nc.dram_tensor(f"zfull_shared{i}", [PSH * P, C], f32,
                                     addr_space="Shared") for i in range(2)]

            # resident constants / metadata
            w1_sb = cp.tile([128, F // 128, HID], f32)
            nc.sync.dma_start(out=w1_sb[:], in_=W1[:].rearrange("(k p) h -> p k h", p=128))
            w2_sb = cp.tile([HID, C], f32)
            nc.sync.dma_start(out=w2_sb[:], in_=W2[:])
            gw_sb = cp.tile([128, NB * G * (CAP + 1)], f32)
            nc.sync.dma_start(out=gw_sb[:], in_=gwd[:])

            z0_sb = cp.tile([128, NB, C], f32)   # alpha-source, node-major permuted
            z_sb = cp.tile([128, NB, C], f32)    # per-iteration z shard

            nc.vector.memset(z0_sb[:], 0.0)
            if 'dve' not in FEAT:
                nc.vector.memset(z_sb[:], 0.0)

            # ---- feature transform: hT = relu(x @ W1).T ; z0 = h @ W2 ----
            if 'xform' in FEAT:
              with tc.tile_pool(name="ht", bufs=1) as hp, \
                 tc.tile_pool(name="xf", bufs=3) as xf, \
                 tc.tile_pool(name="ps", bufs=2, space="PSUM") as ps:
                  hT_sb = hp.tile([HID, NSH], f32)
                  JW = 512
                  for j0 in range(0, NSH, JW):
                      jw = min(JW, NSH - j0)
                      xsb = xf.tile([128, F // 128, JW], f32, tag="xsb")
                      nc.sync.dma_start(
                          out=xsb[:, :, :jw],
                          in_=xT[:, j0:j0 + jw].rearrange("(k p) n -> p k n", p=128))
                      hps = ps.tile([HID, JW], f32, tag="hps")
                      for k in range(F // 128):
                          nc.tensor.matmul(hps[:, :jw], lhsT=w1_sb[:, k, :],
                                           rhs=xsb[:, k, :jw],
                                           start=(k == 0), stop=(k == F // 128 - 1))
                      nc.scalar.activation(out=hT_sb[:, j0:j0 + jw], in_=hps[:, :jw],
                                           func=ActF.Relu)
                  for B in range(NB):
                      pw = 128 if B < NB - 1 else NSH - 128 * (NB - 1)
                      zps = ps.tile([128, C], f32, tag="zps")
                      nc.tensor.matmul(zps[:pw, :], lhsT=hT_sb[:, B * 128:B * 128 + pw],
                                       rhs=w2_sb[:], start=True, stop=True)
                      nc.scalar.copy(out=z0_sb[:pw, B, :], in_=zps[:pw, :])

            # iteration 1 input
            nc.sync.dma_start(out=zag_in[:], in_=z0_sb[:])

            # ---- propagation iterations ----
            with tc.tile_pool(name="msgp", bufs=2) as mp, \
                 tc.tile_pool(name="idxp", bufs=8) as ip, \
                 tc.tile_pool(name="redp", bufs=4) as rp, \
                 tc.tile_pool(name="ohp", bufs=3) as ohp, \
                 tc.tile_pool(name="pf", bufs=2, space="PSUM") as pf:
                for it in range(NITER if 'iters' in FEAT else 0):
                    zfull = zfull2[it % 2]
                    nc.gpsimd.collective_compute(
                        "AllGather", Alu.bypass, replica_groups=rg,
                        ins=[zag_in[:]], outs=[zfull[:]])

                    for sb in range(NSB):
                        nB, base = _sb_geom(sb)
                        msg = mp.tile([128, SBW * G * (CAP + 1), C], f32, tag="msg")
                        for g in range(G):
                            nsl = nB * SEC1
                            w16 = nsl // 16
                            ic = ip.tile([128, SBW * SEC1 // 16], i16, tag=f"ic{g}")
                            col0 = (base + g * nsl) // 16
                            nc.sync.dma_start(out=ic[:, :w16],
                                              in_=gidx[:, col0:col0 + w16])
                            nc.gpsimd.dma_gather(
                                out_ap=msg[:, g * (nB * (CAP + 1)):(g + 1) * (nB * (CAP + 1)), :],
                                in_ap=zfull[g * CHW:(g + 1) * CHW, :],
                                idxs_ap=ic[:, :w16],
                                num_idxs=nsl, num_idxs_reg=nsl, elem_size=C,
                                single_packet=False, queue_num=g)
                        # weight multiply (incl. fixup slots at weight 1.0)
                        if 'dve' in FEAT:
                            wv = gw_sb[:, base // 128:base // 128 + nB * G * (CAP + 1)] \
                                .unsqueeze(2).broadcast_to(
                                    [128, nB * G * (CAP + 1), C])
                            nc.vector.scalar_tensor_tensor(
                                out=msg[:, :nB * G * (CAP + 1), :],
                                in0=msg[:, :nB * G * (CAP + 1), :],
                                scalar=1.0, in1=wv, op0=Alu.mult, op1=Alu.mult)
                        mfull = msg[:, :G * nB * (CAP + 1), :].rearrange(
                            "p (g x) c -> p g x c", g=G)
                        grid = mfull[:, :, :nB * CAP, :].rearrange(
                            "p g (b k) c -> p g b k c", k=CAP)
                        for bl in range(nB):
                            if 'dve' not in FEAT:
                                continue
                            B = sb * SBW + bl
                            red = rp.tile([128, C], f32, tag="red")
                            nc.vector.tensor_reduce(
                                out=red[:], in_=grid[:, :, bl].transpose([0, 3, 1, 2]),
                                axis=Axis.XY, op=Alu.add)
                            nc.vector.scalar_tensor_tensor(
                                out=z_sb[:, B, :], in0=z0_sb[:, B, :], scalar=ALPHA,
                                in1=red[:], op0=Alu.mult, op1=Alu.add)
                        # overflow edges: one-hot matmul reduce on PE
                        if 'fix' not in FEAT:
                            continue
                        oh = ohp.tile([128, SBW * G, 128], f32, tag="oh")
                        nc.sync.dma_start(
                            out=oh[:, :nB * G, :],
                            in_=fohd[:, (sb * SBW) * G * 128:
                                     (sb * SBW + nB) * G * 128].rearrange(
                                         "p (x j) -> p x j", j=128))
                        psF = pf.tile([128, SBW, C], f32, tag="psF")
                        for bl in range(nB):
                            for g in range(G):
                                nc.tensor.matmul(
                                    psF[:, bl, :], lhsT=oh[:, bl * G + g, :],
                                    rhs=mfull[:, g, nB * CAP + bl, :],
                                    start=(g == 0), stop=(g == G - 1))
                        for bl in range(nB):
                            B = sb * SBW + bl
                            nc.vector.scalar_tensor_tensor(
                                out=z_sb[:, B, :], in0=z_sb[:, B, :], scalar=1.0,
                                in1=psF[:, bl, :], op0=Alu.mult, op1=Alu.add)

                    nc.sync.dma_start(out=zag_in[:], in_=z_sb[:])

                # ---- log_softmax on final z ----
                with tc.tile_pool(name="smx", bufs=2) as sp:
                  if 'smx' in FEAT:
                    mneg = cp.tile([128, NB], f32)
                    nc.vector.tensor_reduce(out=mneg[:], in_=z_sb[:],
                                            axis=Axis.X, op=Alu.max)
                    nc.vector.tensor_scalar_mul(mneg[:], mneg[:], -1.0)
                    ssum = cp.tile([128, NB], f32)
                    for B in range(NB):
                        e_scr = sp.tile([128, C], f32, tag="escr")
                        nc.scalar.activation(out=e_scr[:], in_=z_sb[:, B, :],
                                             func=ActF.Exp, bias=mneg[:, B:B + 1],
                                             accum_out=ssum[:, B:B + 1])
                    lneg = cp.tile([128, NB], f32)
                    nc.scalar.activation(out=lneg[:], in_=ssum[:], func=ActF.Ln)
                    nc.vector.tensor_scalar_mul(lneg[:], lneg[:], -1.0)
                    for B in range(NB):
                        nc.vector.tensor_scalar(
                            z0_sb[:, B, :], z_sb[:, B, :],
                            mneg[:, B:B + 1], lneg[:, B:B + 1],
                            Alu.add, Alu.add)

                    nc.sync.dma_start(
                        out=out_p[0:128 * (NB - 1), :].rearrange(
                            "(b p) c -> p b c", p=128),
                        in_=z0_sb[:, 0:NB - 1, :])
                    lastw = NSH - 128 * (NB - 1)
                    nc.sync.dma_start(out=out_p[128 * (NB - 1):NSH, :],
                                      in_=z0_sb[0:lastw, NB - 1, :])

    nc.finalize()
    return nc


def _prow(n):
    """global node id -> permuted z-table row"""
    s, loc = n // NSH, n % NSH
    return s * PSH + (loc % 128) * NB + loc // 128


def _rank_within(key):
    """rank of each element within its group (key values arbitrary)."""
    order = np.argsort(key, kind='stable')
    ks = key[order]
    if len(ks) == 0:
        return order, np.zeros(0, np.int64)
    starts = np.concatenate([[True], ks[1:] != ks[:-1]])
    gid = np.cumsum(starts) - 1
    first = np.full(gid[-1] + 1, len(ks), np.int64)
    np.minimum.at(first, gid, np.arange(len(ks)))
    rank = np.arange(len(ks)) - first[gid]
    return order, rank


def _preprocess(x, edge_index, edge_weight):
    src = edge_index[0].astype(np.int64)
    dst = edge_index[1].astype(np.int64)
    w = edge_weight.astype(np.float32)

    sp = _prow(src)
    g_e = sp // CHW
    lidx = (sp % CHW).astype(np.int64)
    core = dst // NSH
    dloc = dst % NSH

    cell = (core * NSH + dloc) * G + g_e
    order, rank = _rank_within(cell)
    do, wo, go, lo, co = (a[order] for a in (dloc, w, g_e, lidx, core))
    ing = rank < CAP

    B = do // 128
    p = do % 128
    sbb = np.minimum(B // SBW, NSB - 1)
    b_loc = B - sbb * SBW
    nB_arr = np.where(sbb < NSB - 1, SBW, NB - SBW * (NSB - 1))
    sb_base = sbb * (SBW * SEC1 * G)
    sec_base = sb_base + go * (nB_arr * SEC1)

    gidx_all = np.zeros((P, NSLOT), np.int16)
    # per-slot weight columns in msg-tile order (slot//128)
    gw_all = np.zeros((P, 128, NB * G * (CAP + 1)), np.float32)
    m = ing
    i_slot = sec_base[m] + (b_loc[m] * CAP + rank[m]) * 128 + p[m]
    gidx_all[co[m], i_slot] = lo[m].astype(np.int16)
    gw_all[co[m], p[m], i_slot // 128] = wo[m]
    # fixup columns pass through the weight multiply unscaled
    fix_cols = []
    for sb in range(NSB):
        nBq, baseq = _sb_geom(sb)
        for gq in range(G):
            sec0 = baseq // 128 + gq * nBq * (CAP + 1)
            fix_cols.extend(sec0 + nBq * CAP + np.arange(nBq))
    gw_all[:, :, np.asarray(fix_cols)] = 1.0

    # overflow edges -> fixup slots: one 128-slot block per (B, g);
    # one-hot (with weight) matrices streamed to PE at runtime
    foh_all = np.zeros((P, 128, NB * G * 128), np.float32)
    ov = ~ing
    okey = (co[ov] * NB + B[ov]) * G + go[ov]
    oord, orank = _rank_within(okey)
    assert orank.size == 0 or orank.max() < 128, f"fixup overflow {orank.max()}"
    o_c, o_B, o_g = co[ov][oord], B[ov][oord], go[ov][oord]
    o_bl = b_loc[ov][oord]
    o_secb = sec_base[ov][oord]
    o_nB = nB_arr[ov][oord]
    fi_slot = o_secb + (o_nB * CAP + o_bl) * 128 + orank
    gidx_all[o_c, fi_slot] = lo[ov][oord].astype(np.int16)
    foh_all[o_c, orank,
            (o_B * G + o_g) * 128 + (do[ov][oord] % 128)] = wo[ov][oord]

    gidx_w = np.tile(gidx_all.reshape(P, NSLOT // 16, 16).transpose(0, 2, 1),
                     (1, 8, 1))
    return gidx_w, gw_all, foh_all


def _make_in_maps(x, edge_index, edge_weight, W1, W2):
    gidx_w, gw_all, foh_all = _preprocess(x, edge_index, edge_weight)
    in_maps = []
    for c in range(P):
        in_maps.append({
            "xT": np.ascontiguousarray(x[c * NSH:(c + 1) * NSH].T),
            "W1": W1, "W2": W2,
            "gidx": np.ascontiguousarray(gidx_w[c]),
            "gw": np.ascontiguousarray(gw_all[c]),
            "foh": np.ascontiguousarray(foh_all[c]),
        })
    return in_maps


def kernel(x, edge_index, edge_weight, W1, W2):
    x = np.ascontiguousarray(np.asarray(x, np.float32))
    edge_index = np.asarray(edge_index, np.int32)
    edge_weight = np.asarray(edge_weight, np.float32)
    W1 = np.ascontiguousarray(np.asarray(W1, np.float32))
    W2 = np.ascontiguousarray(np.asarray(W2, np.float32))

    if 'nc' not in _NC_CACHE:
        _NC_CACHE['nc'] = _build_graph()
    nc = _NC_CACHE['nc']

    in_maps = _make_in_maps(x, edge_index, edge_weight, W1, W2)
    res = run_bass_kernel_spmd(nc, in_maps, core_ids=list(range(P)))
    out = np.concatenate([res.results[c]["out"] for c in range(P)], axis=0)
    return out.astype(np.float32)
